# revision 40
# baseline (speedup 1.0000x reference)
"""AutoCorrelation Trainium2 kernel (Bass/Tile, 8 NeuronCores) — v3.

Math (per row r of [B*L, 512] with D=512):
  corr_r = irfft(rfft(q_r) * conj(rfft(k_r)))            (circular cross-correlation)
  mean_r = mean(top7(corr_r))
  out = v + sigmoid(mean - corr) * (roll(v,-1,L) - v)

Implementation notes:
  - Host casts q/k/v to fp16 before upload and the kernel emits an fp16
    output (cast back to fp32 on host): HBM traffic 33 MiB/core instead of
    64, and every DMA is HWDGE (no SWDGE casts).
  - DFT/iDFT as fp16 matmuls with a packed-real 512-point basis
    (A-block f=0..255 = Re[f] with A[0]=Re0, B-block = Im[f] with
    B[0]=Re256).  q/k arrive host pre-transposed so forward rhs are plain
    full-rate DMA loads; q/k share each W-block LDWEIGHTS (interleaved).
    PE-only probe of this MM/LDW stream sustains 168us/iter = 97.5% of the
    2.4GHz 1-col/cycle roofline, so PE is the kernel's floor.
  - Product spectrum on DVE fp16 (2x_1P mode), superblock-wide FD=2048 ops
    + a single [1,1024] f=0 fixup (the A0 un-mix is folded into the C
    basis: Cb[0] = (cos(pi t) - 1)/D).
  - Inverse GEMM accumulates into PSUM with C pre-scaled by 1/7, so
    reduce_sum(top7) IS the top-k mean; ACT sigmoid(bias=mean, scale=-7)
    reads PSUM directly.
  - Software pipelining: the last DEFER=6 inverse chunks of superblock i
    are emitted between superblock i+1's forward GEMM groups (1,1,2,2 per
    gap), so their max8/reduce/sigmoid/blend/store consumer chains drain
    while PE has forward work queued, and at most 2 inverse chunks ever run
    back-to-back - the ~2.5us consumer chain never overruns the 2-bank cps
    ring, so the inverse MMs never stall PE (v9 ring 3-deep to keep v(i-1)
    live through its deferred blend).
  - ALL bulk elementwise stays on DVE: GpSimd measures ~2x slower than its
    cost model here and every blend placement on it lost 20-100us.  The
    output store rides the ACT DGE queue; input prefetch owns the SP ring.
  - Row interleave: partition p = row//64, subblock s = row%64 so
    roll(v,-1) = "read subblock s+1"; v is loaded 9 subblocks per
    8-subblock superblock (vnext = v9[:,1:9]); the last superblock's 9th
    slot is filled by a strided row-64k load + 4 batch-wrap rows.
  - Sharding: batch-parallel, 4 batches per core, no communication.
"""
import numpy as np

B, L, D = 32, 2048, 512
N_CORES = 8
BPC = B // N_CORES            # batches per core
ROWS = BPC * L                # 8192 rows per core
NSUB = 64                     # subblocks (s = row % 64)
P = 128                       # partitions (p = row // 64)
SB_GROUP = 8                  # subblocks per superblock
NSUPER = NSUB // SB_GROUP     # 8 superblocks
TOPK = 7

_CACHE = {}


def _dft_consts():
    """Packed-real DFT matrices W [512 feat, 512 packed] and C [512 packed, 512 t].
    C is pre-scaled by 1/TOPK so sum(top7(corr')) == mean(top7(corr))."""
    j = np.arange(D)[:, None].astype(np.float64)
    f = np.arange(256)[None, :].astype(np.float64)
    Wc = np.cos(-2 * np.pi * j * f / D)
    Ws = np.sin(-2 * np.pi * j * f / D)
    WB = Ws.copy()
    WB[:, 0] = np.cos(np.pi * j[:, 0])          # B0 row: Re256
    W = np.concatenate([Wc, WB], axis=1)        # [512, 512]
    t = np.arange(D)[None, :].astype(np.float64)
    fc = np.arange(256)[:, None].astype(np.float64)
    # Packed-real f=0 slots: A0 carries s = Q0*K0 + Q256*K256 (what the
    # generic complex-product ops produce there, no fixup), and B0 is
    # overwritten by the one-mul fixup with b = Q256*K256.  Basis rows:
    # corr += s*Ca0 + b*(Cb0 - Ca0) = Q0*K0*Ca0 + Q256*K256*Cb0 exactly.
    Ca = np.cos(2 * np.pi * fc * t / D) * 2 / D
    Ca[0] = 1.0 / D
    Cb = -np.sin(2 * np.pi * fc * t / D) * 2 / D
    Cb[0] = (np.cos(np.pi * t[0]) - 1.0) / D
    C = np.concatenate([Ca, Cb], axis=0) / TOPK  # [512, 512]
    return W.astype(np.float32), C.astype(np.float32)


def _build_nc(n_iter=1, n_super=NSUPER, o16_mode="dve"):
    import concourse.bacc as bacc
    import concourse.mybir as mybir
    from concourse.tile import TileContext

    f16 = mybir.dt.float16
    f32 = mybir.dt.float32

    W, C = _dft_consts()
    # W16[p, jj, fp]  = W[jj*128+p, fp]   (lhsT blocks for GEMM-1)
    W16 = W.reshape(4, P, D).transpose(1, 0, 2).astype(np.float16).copy()
    # C16[p, ff, t]   = C[ff*128+p, t]    (rhs blocks for GEMM-2)
    C16 = C.reshape(4, P, D).transpose(1, 0, 2).astype(np.float16).copy()

    nc = bacc.Bacc()
    # query/key arrive HOST-PRE-TRANSPOSED as [a, sbi, jj, s, r]:
    # element = q[row 64*r + 8*sbi + s, feat jj*128+a].  Each superblock's
    # slice [:, sbi] is one contiguous 8 KiB/partition block, so the
    # forward-DFT rhs is plain full-rate DMA loads (no xbar transposes).
    q_d = nc.dram_tensor("query", [P, NSUPER, 4, SB_GROUP, P], f16,
                         kind="ExternalInput")
    k_d = nc.dram_tensor("key", [P, NSUPER, 4, SB_GROUP, P], f16,
                         kind="ExternalInput")
    v_d = nc.dram_tensor("value", [ROWS, D], f16, kind="ExternalInput")
    o_d = nc.dram_tensor("out", [ROWS, D], f16, kind="ExternalOutput")
    w_t = nc.inline_tensor(W16, name="Wdft")
    c_t = nc.inline_tensor(C16, name="Cdft")

    # interleaved views: [p, s, c] with row = 64*p + s
    vv = v_d.rearrange("(p s) c -> p s c", s=NSUB)
    ov = o_d.rearrange("(p s) c -> p s c", s=NSUB)

    with TileContext(nc) as tc:
        with (
            tc.tile_pool(name="consts", bufs=1) as consts,
            tc.tile_pool(name="io", bufs=2) as io,
            tc.tile_pool(name="work", bufs=2) as work,
            tc.tile_pool(name="small", bufs=8) as small,
            tc.tile_pool(name="ps", bufs=3, space="PSUM") as psp,
            tc.tile_pool(name="pscb", bufs=2, space="PSUM") as pscp,
        ):
            wt = consts.tile([P, 4, D], f16)      # W16
            ct = consts.tile([P, 4, D], f16)      # C16
            nc.sync.dma_start(out=wt, in_=w_t[:, :, :])
            # ct is loaded inside superblock 0 (n_iter=1), after its input
            # DMAs: the inverse GEMM doesn't need it until ~20us in, and this
            # keeps the first q/k transposes at the head of the SP ring.

            # dummy sigmoid: forces the one ACT table-set load into the fill
            # window (otherwise it stalls the pipe at the first real sigmoid)
            warm = small.tile([1, 8], f16, tag="warm")
            nc.scalar.activation(warm, wt[0:1, 0, 0:8],
                                 mybir.ActivationFunctionType.Sigmoid,
                                 scale=-1.0)

            H = SB_GROUP // 2
            DEFER = 6   # inverse chunks of sb i that run inside sb i+1
            # per-forward-gap chunk counts: later gaps take 2 so only 2
            # chunks run back-to-back at sb end (cps ring 2 never stalls PE)
            GAP_CHUNKS = ((2,), (3,), (4, 5), (6, 7))

            def inv_chunk(st, ch):
                """Inverse GEMM + top-k/sigmoid consumers for one 128-row
                chunk; emits the blend when a half completes and the store
                after the last chunk."""
                cps = pscp.tile([P, D], f32, tag="psc1bank")
                for ff in range(4):
                    nc.tensor.matmul(cps, st["pt"][:, ff, ch * P:(ch + 1) * P],
                                     ct[:, ff, :], start=(ff == 0),
                                     stop=(ff == 3))
                mx = small.tile([P, 8], f32, tag="mx")
                nc.vector.max(out=mx, in_=cps)
                pm = small.tile([P, 1], f32, tag="pm")
                nc.vector.reduce_sum(pm, mx[:, 0:TOPK],
                                     axis=mybir.AxisListType.X)
                nc.scalar.activation(st["w1"][:, ch, :], cps,
                                     mybir.ActivationFunctionType.Sigmoid,
                                     bias=pm, scale=-float(TOPK))
                if ch % H == H - 1:
                    h = ch // H
                    hs = slice(h * H, (h + 1) * H)
                    nc.vector.tensor_mul(st["zt"][:, hs, :],
                                         st["w1"][:, hs, :], st["dt"][:, hs, :])
                    on_pool = (o16_mode == "pool"
                               or (o16_mode == "split" and h == 0))
                    eng = nc.gpsimd if on_pool else nc.vector
                    eng.tensor_add(st["o16"][:, hs, :], st["v9"][:, hs, :],
                                   st["zt"][:, hs, :])
                if ch == SB_GROUP - 1:
                    # store rides the ACT DGE queue so the SP ring stays
                    # dedicated to input prefetch
                    nc.scalar.dma_start(out=ov[:, st["sl"], :], in_=st["o16"])

            def superblock(sbi, prev):
                sl = slice(sbi * SB_GROUP, (sbi + 1) * SB_GROUP)
                # qT8[a, jj, s, r] = q[row 64r + 8sbi + s, jj*128+a]
                # (3-deep rings: loads run a full superblock ahead)
                qT8 = work.tile([P, 4, SB_GROUP, P], f16, tag="qT8", bufs=3)
                kT8 = work.tile([P, 4, SB_GROUP, P], f16, tag="kT8", bufs=3)
                for jh in range(2):
                    js = slice(2 * jh, 2 * jh + 2)
                    nc.sync.dma_start(out=qT8[:, js, :, :],
                                      in_=q_d[:, sbi, js, :, :])
                    nc.sync.dma_start(out=kT8[:, js, :, :],
                                      in_=k_d[:, sbi, js, :, :])

                # v9 is 3-deep: v9(i-1) stays live through its deferred blend
                # mid-sb i while v9(i+1) prefetches
                v9 = io.tile([P, SB_GROUP + 1, D], f16, tag="v9", bufs=3)
                if sbi < NSUPER - 1:
                    nc.sync.dma_start(
                        out=v9, in_=vv[:, sbi * SB_GROUP:(sbi + 1) * SB_GROUP + 1, :])
                else:
                    nc.sync.dma_start(out=v9[:, 0:SB_GROUP, :], in_=vv[:, sl, :])
                    # v9[p, 8] = v[row 64p+64]; wraps at p in {31,63,95,127}
                    nc.sync.dma_start(
                        out=v9[0:127, SB_GROUP, :],
                        in_=v_d.rearrange("(a b) c -> a b c", b=NSUB)[1:128, 0])
                    nc.sync.dma_start(
                        out=v9.rearrange("(w u) s c -> w u s c", u=32)[:, 31, SB_GROUP, :],
                        in_=v_d.rearrange("(b t) c -> b t c", t=L)[:, 0, :])

                if sbi == 0 and n_iter == 1:
                    nc.sync.dma_start(out=ct, in_=c_t[:, :, :])

                # dt = roll(v,-1) - v, on DVE, first thing in its queue for
                # this superblock: v9 is prefetched a superblock ahead, so dt
                # has no upstream dependency and fills DVE's early-sb idle.
                # (GpSimd measures ~2x slower than its cost model on these
                # ops, and every placement trial with it came back slower.)
                dt_ = work.tile([P, SB_GROUP, D], f16, tag="dt")
                nc.vector.tensor_sub(dt_, v9[:, 1:SB_GROUP + 1, :],
                                     v9[:, 0:SB_GROUP, :])

                # forward DFT with N=512 moving operands (4 subblocks per
                # group-half gh, 2 freq-chunks per PSUM tile): half the MM and
                # LDWEIGHTS count of an N=256 scheme for the same streamed
                # columns; q/k share each W-block LDWEIGHTS
                # forward groups, with the previous superblock's deferred
                # inverse chunks slotted between them: their consumer chains
                # (max/sigmoid/blend/store) drain while PE has forward work
                # queued, instead of colliding at the superblock boundary
                qf = work.tile([P, 4, 4 * 256], f16, tag="qf")
                kf = work.tile([P, 4, 4 * 256], f16, tag="kf")
                for gi, (gh, mh) in enumerate(((0, 0), (0, 1), (1, 0), (1, 1))):
                    psq = psp.tile([P, 2, 512], f32, tag="ps2bank")
                    psk = psp.tile([P, 2, 512], f32, tag="ps2bank")
                    for mi in range(2):
                        mm = 2 * mh + mi
                        for jj in range(4):
                            lw = wt[:, jj, mm * P:(mm + 1) * P]
                            rq = qT8[:, jj, 4 * gh:4 * gh + 4, :]
                            rk = kT8[:, jj, 4 * gh:4 * gh + 4, :]
                            nc.tensor.matmul(psq[:, mi, :], lw, rq,
                                             start=(jj == 0), stop=(jj == 3))
                            nc.tensor.matmul(psk[:, mi, :], lw, rk,
                                             start=(jj == 0), stop=(jj == 3))
                    nc.scalar.copy(
                        qf[:, 2 * mh:2 * mh + 2, gh * 512:(gh + 1) * 512], psq)
                    nc.scalar.copy(
                        kf[:, 2 * mh:2 * mh + 2, gh * 512:(gh + 1) * 512], psk)
                    if prev is not None:
                        for dch in GAP_CHUNKS[gi]:
                            inv_chunk(prev, dch)

                # product spectrum P = QF o conj(KF) on DVE, full superblock
                # width per op (2048 elem) to amortize the fixed per-inst
                # latency; pt is 3-deep so next-superblock products don't wait
                # for this superblock's inverse to drain it
                pt = work.tile([P, 4, 1024], f16, tag="pt", bufs=3)
                QA, QB = qf[:, 0:2, :], qf[:, 2:4, :]
                KA, KB = kf[:, 0:2, :], kf[:, 2:4, :]
                T1 = work.tile([P, 2, 1024], f16, tag="t1", bufs=1)
                T2 = work.tile([P, 2, 1024], f16, tag="t2", bufs=1)
                nc.vector.tensor_mul(T1, QA, KA)
                nc.vector.tensor_mul(T2, QB, KB)
                nc.vector.tensor_add(pt[:, 0:2, :], T1, T2)
                nc.vector.tensor_mul(T1, QB, KA)
                nc.vector.tensor_mul(T2, QA, KB)
                nc.vector.tensor_sub(pt[:, 2:4, :], T1, T2)
                # f=0 fixup: only B0 := Q256*K256 (A0 keeps the generic
                # product's Q0K0+Q256K256; the C basis un-mixes it)
                nc.vector.tensor_mul(
                    pt[0:1, 2, :], qf[0:1, 2, :], kf[0:1, 2, :])

                # inverse DFT per 128-row chunk (= subblock), then w1 weights;
                # the last DEFER chunks run inside the next superblock
                w1sb = work.tile([P, SB_GROUP, D], f16, tag="w1sb")
                zt = work.tile([P, SB_GROUP, D], f16, tag="zt", bufs=1)
                o16 = io.tile([P, SB_GROUP, D], f16, tag="o16")
                st = {"sl": sl, "v9": v9, "dt": dt_, "pt": pt,
                      "w1": w1sb, "zt": zt, "o16": o16}
                for ch in range(SB_GROUP - DEFER):
                    inv_chunk(st, ch)
                return st

            def pipeline():
                prev = None
                for sbi in range(n_super):
                    prev = superblock(sbi, prev)
                for ch in range(SB_GROUP - DEFER, SB_GROUP):
                    inv_chunk(prev, ch)

            if n_iter == 1:
                pipeline()
            else:
                nc.sync.dma_start(out=ct, in_=c_t[:, :, :])
                with tc.For_i(0, n_iter, 1):
                    pipeline()

    nc.finalize()
    return nc


def kernel(query, key, value):
    import sys
    if "/opt/trn_rl_repo" not in sys.path:
        sys.path.insert(0, "/opt/trn_rl_repo")
    from concourse.bass_utils import run_bass_kernel_spmd

    if "nc" not in _CACHE:
        _CACHE["nc"] = _build_nc()
    nc = _CACHE["nc"]

    q = np.asarray(query, dtype=np.float32).reshape(B, L, D).astype(np.float16)
    k = np.asarray(key, dtype=np.float32).reshape(B, L, D).astype(np.float16)
    v = np.asarray(value, dtype=np.float32).reshape(B, L, D).astype(np.float16)

    def pre_t(x, c):
        # [ROWS, D] -> [a, sbi, jj, s, r]: out = x[64*r + 8*sbi + s, jj*128+a]
        xc = x[c * BPC:(c + 1) * BPC].reshape(ROWS, D)
        return np.ascontiguousarray(
            xc.reshape(P, NSUPER, SB_GROUP, 4, P).transpose(4, 1, 3, 2, 0))

    in_maps = []
    for c in range(N_CORES):
        sl = slice(c * BPC, (c + 1) * BPC)
        in_maps.append({
            "query": pre_t(q, c),
            "key": pre_t(k, c),
            "value": np.ascontiguousarray(v[sl].reshape(ROWS, D)),
        })
    res = run_bass_kernel_spmd(nc, in_maps, core_ids=list(range(N_CORES)),
                               trace=bool(_CACHE.get("trace")))
    _CACHE["last_result"] = res
    out = np.empty((B, L, D), dtype=np.float32)
    for c in range(N_CORES):
        out[c * BPC:(c + 1) * BPC] = res.results[c]["out"].astype(
            np.float32).reshape(BPC, L, D)
    return out

